# revision 18
# baseline (speedup 1.0000x reference)
"""ColumnParallelLinearWithMoE Trainium2 kernel.

Expert-parallel: expert e -> NeuronCore e. Each core computes
    y_e [8192, 512] = x_e [8192, 1024] @ W_e.T [1024, 512] + b_e
where x_e = input_[idx_list[e]] flattened over (per, seq).

Routing gather/scatter, the x transpose (contraction dim onto SBUF
partitions) and all layout packing happen on the host; the device does the
dense matmul at the PE roofline (512 MMs x 216ns ~= 110.6us warm).

Ramp design (the stream itself is already at roofline):
 - x pieces go out on the sync HWDGE ring, w pair-chunks on the scalar
   ring, so the two load chains fill SBUF concurrently.
 - Every DMA piece is a flat contiguous column-slice (>=1KB descriptor
   elements) and the early piece count stays small: HWDGE completion
   semaphores are a shared pool of ~8; overrunning them serializes issues.
 - Super 0 is computed k-across-j in data-availability order over all 8
   PSUM banks, so the PE consumes (k, j) matmuls the moment w and x pieces
   land instead of serializing on the full 1MB w chain.
 - A short tunable warm-up MM stream (no DMA deps) bridges the first
   ~2.4us so HAM un-throttles (1.2 -> 2.4 GHz) right as real data lands.
Tail: y stored as bf16 (host widens to fp32), last super stored per
128-token group and the final group split into 2x256-column accumulations
so the last store is small and leaves early.
"""

import sys

if "/opt/trn_rl_repo" not in sys.path:
    sys.path.insert(0, "/opt/trn_rl_repo")

import numpy as np

# Problem constants (hardcoded per harness contract).
E = 8
BS = 64
S = 1024
D = 1024
OPP = 512
P = 128
TOK = (BS // E) * S  # 8192 tokens per expert
KT = D // P          # 8 contraction tiles
TW = 1024            # token-superblock width staged in SBUF
NSUP = TOK // TW
TPS = TW // P        # token tiles (of 128) per superblock
JBLK = KT * P        # elements per j-block in the super-0 packing

VARIANT = "bf16"

# Warm-up matmuls (N=128, ~107ns each cold): span must bridge preamble end
# to first-data-landed (~2.8us later).
NWARM = 54
# Data-gated filler MMs (dep on the first w/x pieces only): keep the PE busy
# inside the ramp's DMA-wait holes so HAM never re-throttles.
NFILL = 10

_programs: dict[str, tuple] = {}


def _super0_order():
    """(k, j) matmul order for super 0, sorted by predicted data arrival.

    w k-pair chunks land ~1.1us apart on the scalar ring; x pieces land on
    the sync ring (j0 split in k-halves, then one piece per j).  Emitting
    the MMs in availability order keeps the PE fed from the first piece.
    """
    def x_t(k, j):
        if j == 0:
            return -0.8 if k < 2 else (-0.1 if k < 4 else 0.8)
        return [None, 0.3, 1.3, 2.0, 3.2, 4.4, 5.6, 6.8][j]

    def w_t(k):
        return [0.0, 0.8, 2.1, 2.1, 2.4, 2.4, 3.1, 3.1][k]

    order = sorted(
        ((k, j) for k in range(KT) for j in range(TPS)),
        key=lambda kj: (max(w_t(kj[0]), x_t(kj[0], kj[1])), kj[1], kj[0]),
    )
    # sanity: per j, k must be ascending (PSUM start/stop correctness)
    seen = {}
    for k, j in order:
        assert seen.get(j, -1) == k - 1, (k, j)
        seen[j] = k
    return order


def _build(variant: str):
    import concourse.bacc as bacc
    import concourse.tile as tile
    from concourse import mybir

    if variant == "f32":
        mm_dt = mybir.dt.float32
        np_in = np.float32
    elif variant == "f32r":
        mm_dt = mybir.dt.float32r
        np_in = np.float32
    elif variant == "bf16":
        import ml_dtypes

        mm_dt = mybir.dt.bfloat16
        np_in = ml_dtypes.bfloat16
    else:
        raise ValueError(variant)

    nc = bacc.Bacc(None, target_bir_lowering=False, debug=False)

    # DRAM tensors, all host-packed so every DMA piece is one contiguous
    # column-slice per partition.
    # x: slab 0 is super 0 in [p][j][k][u] order; slabs 1..7 are supers in
    # [p][k][t] order.
    x = nc.dram_tensor("x", [NSUP, P, KT * TW], mm_dt, kind="ExternalInput")
    # w: [p][k][c]; k-pair pieces are 2KB runs per partition.
    w = nc.dram_tensor("w", [P, KT * OPP], mm_dt, kind="ExternalInput")
    bias = nc.dram_tensor("bias", [P, OPP], mybir.dt.float32, kind="ExternalInput")
    # y: [p][s][j][c] for supers 0..6 then [p][j][c] for super 7.
    y = nc.dram_tensor("y", [P, NSUP * TPS * OPP], mm_dt, kind="ExternalOutput")
    YG0 = (NSUP - 1) * TPS * OPP  # column offset of super-7 groups

    with tile.TileContext(nc) as tc:
        with (
            tc.tile_pool(name="wpool", bufs=1) as wpool,
            tc.tile_pool(name="bpool", bufs=1) as bpool,
            tc.tile_pool(name="xpool", bufs=4) as xpool,
            tc.tile_pool(name="opool", bufs=2) as opool,
            tc.tile_pool(name="ogpool", bufs=4) as ogpool,
            tc.tile_pool(name="pspool", bufs=8, space="PSUM") as pspool,
        ):
            # ---- PE warm-up: N=128 MMs with no DMA deps, bridging the DMA
            # latency window so HAM is at 2.4GHz when real data lands.
            warm_src = wpool.tile([P, P], mybir.dt.bfloat16, tag="warm")
            nc.gpsimd.memset(warm_src[:], 0.0)
            warm_ps = pspool.tile([P, OPP], mybir.dt.float32, name="ps")
            for _ in range(NWARM):
                nc.tensor.matmul(
                    warm_ps[:, 0:P], warm_src[:], warm_src[:], start=True, stop=True
                )

            # ---- load issue: x chain on sync ring, w chain on scalar ring.
            # Super 0 staged in [p][j][k][u] order; flat views keep each DMA
            # one contiguous run per partition.
            x0_sb = xpool.tile([P, TPS, KT, P], mm_dt, tag="x0")
            x0f = x0_sb.rearrange("p j k u -> p (j k u)")
            Q = JBLK // 4
            nc.sync.dma_start(out=x0f[:, 0:Q], in_=x[0][:, 0:Q])
            nc.sync.dma_start(out=x0f[:, Q : 2 * Q], in_=x[0][:, Q : 2 * Q])
            nc.sync.dma_start(out=x0f[:, 2 * Q : JBLK], in_=x[0][:, 2 * Q : JBLK])
            for j in range(1, TPS):
                # j1/j2 ride the third (gpsimd SWDGE) queue so the sync ring
                # reaches the later pieces sooner during the ramp.
                eng = nc.gpsimd if j in (1, 2) else nc.sync
                eng.dma_start(
                    out=x0f[:, j * JBLK : (j + 1) * JBLK],
                    in_=x[0][:, j * JBLK : (j + 1) * JBLK],
                )

            # w chain: w0/w1/w45 on the scalar ring, w23/w67 on the gpsimd
            # queue (behind the xj1/xj2 issues), so all of w lands early.
            w_sb = wpool.tile([P, KT, OPP], mm_dt)
            wf = w_sb.rearrange("p k c -> p (k c)")
            for lo, hi, eng in (
                (0, 1, nc.scalar),
                (1, 2, nc.scalar),
                (2, 4, nc.gpsimd),
                (4, 6, nc.scalar),
                (6, 8, nc.gpsimd),
            ):
                eng.dma_start(
                    out=wf[:, lo * OPP : hi * OPP], in_=w[:, lo * OPP : hi * OPP]
                )
            bias_sb = bpool.tile([P, OPP], mybir.dt.float32)
            nc.scalar.dma_start(out=bias_sb[:], in_=bias[:])

            # supers 1..7: one big contiguous DMA each on the sync ring.
            xs_sb = [None]
            for s in range(1, NSUP):
                t = xpool.tile([P, KT, TW], mm_dt, tag="x")
                nc.sync.dma_start(
                    out=t.rearrange("p k t -> p (k t)"), in_=x[s]
                )
                xs_sb.append(t)

            # ---- super 0: availability-ordered (k, j) over all 8 banks.
            # (The 8th accumulator recycles the warm-up bank; the WAW dep on
            # the in-order PE is free.)
            ps0 = [
                pspool.tile([P, OPP], mybir.dt.float32, name="ps")
                for j in range(TPS)
            ]
            o_sb = opool.tile([P, TPS, OPP], mm_dt, tag="o")
            first = True
            for k, j in _super0_order():
                nc.tensor.matmul(
                    ps0[j][:],
                    x0_sb[:, j, k, :],
                    w_sb[:, k, :],
                    start=(k == 0),
                    stop=(k == KT - 1),
                )
                if first:
                    # ramp fillers: gated on the same first pieces, so they
                    # execute inside the later pieces' DMA-wait holes.
                    for _ in range(NFILL):
                        nc.tensor.matmul(
                            warm_ps[:, 0:P],
                            x0_sb[:, 0, 0, :],
                            warm_src[:],
                            start=True,
                            stop=True,
                        )
                    first = False
            for j in range(TPS):
                nc.vector.tensor_add(o_sb[:, j, :], ps0[j][:], bias_sb[:])
            nc.scalar.dma_start(
                out=y[:, 0 : TPS * OPP],
                in_=o_sb.rearrange("p j c -> p (j c)"),
            )

            # ---- supers 1..6: j-groups, per-super store.
            for s in range(1, NSUP - 1):
                x_sb = xs_sb[s]
                o_sb = opool.tile([P, TPS, OPP], mm_dt, tag="o")
                for j in range(TPS):
                    ps = pspool.tile([P, OPP], mybir.dt.float32, name="ps")
                    for k in range(KT):
                        nc.tensor.matmul(
                            ps[:],
                            x_sb[:, k, j * P : (j + 1) * P],
                            w_sb[:, k, :],
                            start=(k == 0),
                            stop=(k == KT - 1),
                        )
                    nc.vector.tensor_add(o_sb[:, j, :], ps[:], bias_sb[:])
                nc.scalar.dma_start(
                    out=y[:, s * TPS * OPP : (s + 1) * TPS * OPP],
                    in_=o_sb.rearrange("p j c -> p (j c)"),
                )

            # ---- super 7: per-group stores; final group split in 2x256 cols
            # so the very last store is small and leaves early.
            # Stores go on the sync ring here: it is idle by now, and the
            # last store must not queue behind scalar-ring issue latency.
            s = NSUP - 1
            x_sb = xs_sb[s]
            for j in range(TPS):
                og = ogpool.tile([P, OPP], mm_dt, tag="og")
                ps = pspool.tile([P, OPP], mybir.dt.float32, name="ps")
                for k in range(KT):
                    nc.tensor.matmul(
                        ps[:],
                        x_sb[:, k, j * P : (j + 1) * P],
                        w_sb[:, k, :],
                        start=(k == 0),
                        stop=(k == KT - 1),
                    )
                if j < TPS - 1:
                    nc.vector.tensor_add(og[:], ps[:], bias_sb[:])
                    nc.sync.dma_start(
                        out=y[:, YG0 + j * OPP : YG0 + (j + 1) * OPP], in_=og[:]
                    )
                else:
                    # final group: evict + store column halves on both rings
                    # so the two issue latencies overlap.
                    H = OPP // 2
                    for h, eng in ((0, nc.scalar), (1, nc.sync)):
                        nc.vector.tensor_add(
                            og[:, h * H : (h + 1) * H],
                            ps[:, h * H : (h + 1) * H],
                            bias_sb[:, h * H : (h + 1) * H],
                        )
                        eng.dma_start(
                            out=y[
                                :,
                                YG0 + j * OPP + h * H : YG0 + j * OPP + (h + 1) * H,
                            ],
                            in_=og[:, h * H : (h + 1) * H],
                        )

    nc.compile()
    return nc, np_in


def _get_program(variant: str):
    if variant not in _programs:
        _programs[variant] = _build(variant)
    return _programs[variant]


def kernel(input_, idx_list, W, b, **_ignored):
    from concourse.bass_utils import run_bass_kernel_spmd

    input_ = np.asarray(input_)
    idx = np.asarray(idx_list).astype(np.int64)
    W = np.asarray(W, dtype=np.float32)
    b = np.asarray(b, dtype=np.float32)

    nc, np_in = _get_program(VARIANT)

    in_maps = []
    for e in range(E):
        xg = input_[idx[e]].reshape(TOK, D).astype(np.float32, copy=False)
        xt = np.ascontiguousarray(xg.T).astype(np_in)  # [D, TOK]
        a = xt.reshape(KT, P, NSUP, TW)
        xpk = np.empty((NSUP, P, KT * TW), dtype=np_in)
        # super 0 -> [p][j][k][u]
        xpk[0] = (
            a[:, :, 0].reshape(KT, P, TPS, P).transpose(1, 2, 0, 3)
            .reshape(P, KT * TW)
        )
        # supers 1..7 -> [p][k][t]
        xpk[1:] = a.transpose(2, 1, 0, 3)[1:].reshape(NSUP - 1, P, KT * TW)
        # w -> [p][k][c]
        wt = np.ascontiguousarray(
            W[e].T.reshape(KT, P, OPP).transpose(1, 0, 2)
        ).astype(np_in).reshape(P, KT * OPP)
        bias_bc = np.ascontiguousarray(
            np.broadcast_to(b[e][None, :], (P, OPP))
        ).astype(np.float32)
        in_maps.append({"x": xpk, "w": wt, "bias": bias_bc})

    res = run_bass_kernel_spmd(nc, in_maps, core_ids=list(range(E)))

    out = np.zeros((BS, S, E * OPP), dtype=input_.dtype)
    for e in range(E):
        yw = np.asarray(res.results[e]["y"])  # [P, NSUP*TPS*OPP] bf16
        y_main = (
            yw[:, : (NSUP - 1) * TPS * OPP]
            .reshape(P, NSUP - 1, TPS, OPP)
            .transpose(1, 2, 0, 3)
            .reshape((NSUP - 1) * TW, OPP)
        )
        y_last = (
            yw[:, (NSUP - 1) * TPS * OPP :]
            .reshape(P, TPS, OPP)
            .transpose(1, 0, 2)
            .reshape(TW, OPP)
        )
        ye = np.concatenate([y_main, y_last], axis=0).astype(np.float32)
        out[idx[e], :, e * OPP : (e + 1) * OPP] = ye.reshape(BS // E, S, OPP)
    return out


# revision 19
# speedup vs baseline: 1.0352x; 1.0352x over previous
"""ColumnParallelLinearWithMoE Trainium2 kernel.

Expert-parallel: expert e -> NeuronCore e. Each core computes
    y_e [8192, 512] = x_e [8192, 1024] @ W_e.T [1024, 512] + b_e
where x_e = input_[idx_list[e]] flattened over (per, seq).

Routing gather/scatter, the x transpose (contraction dim onto SBUF
partitions) and all layout packing happen on the host; the device does the
dense matmul at the PE roofline (512 MMs x 216ns ~= 110.6us warm).

Ramp design (the stream itself is already at roofline):
 - x pieces go out on the sync HWDGE ring, w pair-chunks on the scalar
   ring, so the two load chains fill SBUF concurrently.
 - Every DMA piece is a flat contiguous column-slice (>=1KB descriptor
   elements) and the early piece count stays small: HWDGE completion
   semaphores are a shared pool of ~8; overrunning them serializes issues.
 - Super 0 is computed k-across-j in data-availability order over all 8
   PSUM banks, so the PE consumes (k, j) matmuls the moment w and x pieces
   land instead of serializing on the full 1MB w chain.
 - A short tunable warm-up MM stream (no DMA deps) bridges the first
   ~2.4us so HAM un-throttles (1.2 -> 2.4 GHz) right as real data lands.
Tail: y stored as bf16 (host widens to fp32), last super stored per
128-token group and the final group split into 2x256-column accumulations
so the last store is small and leaves early.
"""

import sys

if "/opt/trn_rl_repo" not in sys.path:
    sys.path.insert(0, "/opt/trn_rl_repo")

import numpy as np

# Problem constants (hardcoded per harness contract).
E = 8
BS = 64
S = 1024
D = 1024
OPP = 512
P = 128
TOK = (BS // E) * S  # 8192 tokens per expert
KT = D // P          # 8 contraction tiles
TW = 1024            # token-superblock width staged in SBUF
NSUP = TOK // TW
TPS = TW // P        # token tiles (of 128) per superblock
JBLK = KT * P        # elements per j-block in the super-0 packing

VARIANT = "bf16"

# Warm-up matmuls (N=128, ~107ns each cold): span must bridge preamble end
# to first-data-landed (~2.8us later).
NWARM = 27
# Data-gated filler MMs (dep on the first w/x pieces only): keep the PE busy
# inside the ramp's DMA-wait holes so HAM never re-throttles.
NFILL = 28

_programs: dict[str, tuple] = {}


def _super0_order():
    """(k, j) matmul order for super 0, sorted by predicted data arrival.

    w k-pair chunks land ~1.1us apart on the scalar ring; x pieces land on
    the sync ring (j0 split in k-halves, then one piece per j).  Emitting
    the MMs in availability order keeps the PE fed from the first piece.
    """
    def x_t(k, j):
        if j == 0:
            return -0.6 if k < 2 else (0.0 if k < 4 else 1.6)
        return [None, 0.3, 1.3, 2.2, 3.3, 4.4, 5.5, 6.6][j]

    def w_t(k):
        return [0.0, 0.7, 1.9, 1.9, 3.1, 3.1, 4.3, 4.3][k]

    order = sorted(
        ((k, j) for k in range(KT) for j in range(TPS)),
        key=lambda kj: (max(w_t(kj[0]), x_t(kj[0], kj[1])), kj[1], kj[0]),
    )
    # sanity: per j, k must be ascending (PSUM start/stop correctness)
    seen = {}
    for k, j in order:
        assert seen.get(j, -1) == k - 1, (k, j)
        seen[j] = k
    return order


def _build(variant: str):
    import concourse.bacc as bacc
    import concourse.tile as tile
    from concourse import mybir

    if variant == "f32":
        mm_dt = mybir.dt.float32
        np_in = np.float32
    elif variant == "f32r":
        mm_dt = mybir.dt.float32r
        np_in = np.float32
    elif variant == "bf16":
        import ml_dtypes

        mm_dt = mybir.dt.bfloat16
        np_in = ml_dtypes.bfloat16
    else:
        raise ValueError(variant)

    nc = bacc.Bacc(None, target_bir_lowering=False, debug=False)

    # DRAM tensors, all host-packed so every DMA piece is one contiguous
    # column-slice per partition.
    # x: slab 0 is super 0 in [p][j][k][u] order; slabs 1..7 are supers in
    # [p][k][t] order.
    x = nc.dram_tensor("x", [NSUP, P, KT * TW], mm_dt, kind="ExternalInput")
    # w: [p][k][c]; k-pair pieces are 2KB runs per partition.
    w = nc.dram_tensor("w", [P, KT * OPP], mm_dt, kind="ExternalInput")
    bias = nc.dram_tensor("bias", [P, OPP], mybir.dt.float32, kind="ExternalInput")
    # y: [p][s][j][c] for supers 0..6 then [p][j][c] for super 7.
    y = nc.dram_tensor("y", [P, NSUP * TPS * OPP], mm_dt, kind="ExternalOutput")
    YG0 = (NSUP - 1) * TPS * OPP  # column offset of super-7 groups

    with tile.TileContext(nc) as tc:
        with (
            tc.tile_pool(name="wpool", bufs=1) as wpool,
            tc.tile_pool(name="bpool", bufs=1) as bpool,
            tc.tile_pool(name="xpool", bufs=4) as xpool,
            tc.tile_pool(name="opool", bufs=2) as opool,
            tc.tile_pool(name="ogpool", bufs=4) as ogpool,
            tc.tile_pool(name="pspool", bufs=8, space="PSUM") as pspool,
        ):
            # ---- PE warm-up: N=128 MMs with no DMA deps, bridging the DMA
            # latency window so HAM is at 2.4GHz when real data lands.
            warm_src = wpool.tile([P, P], mybir.dt.bfloat16, tag="warm")
            nc.gpsimd.memset(warm_src[:], 0.0)
            warm_ps = pspool.tile([P, OPP], mybir.dt.float32, name="ps")
            for _ in range(NWARM):
                nc.tensor.matmul(
                    warm_ps[:, 0:P], warm_src[:], warm_src[:], start=True, stop=True
                )

            # ---- load issue: x chain on sync ring, w chain on scalar ring.
            # Super 0 staged in [p][j][k][u] order; flat views keep each DMA
            # one contiguous run per partition.
            x0_sb = xpool.tile([P, TPS, KT, P], mm_dt, tag="x0")
            x0f = x0_sb.rearrange("p j k u -> p (j k u)")
            Q = JBLK // 4
            nc.sync.dma_start(out=x0f[:, 0:Q], in_=x[0][:, 0:Q])
            nc.sync.dma_start(out=x0f[:, Q : 2 * Q], in_=x[0][:, Q : 2 * Q])
            nc.sync.dma_start(out=x0f[:, 2 * Q : JBLK], in_=x[0][:, 2 * Q : JBLK])
            for j in range(1, TPS):
                # j1/j2 ride the third (gpsimd SWDGE) queue so the sync ring
                # reaches the later pieces sooner during the ramp.
                eng = nc.gpsimd if j in (1, 2) else nc.sync
                eng.dma_start(
                    out=x0f[:, j * JBLK : (j + 1) * JBLK],
                    in_=x[0][:, j * JBLK : (j + 1) * JBLK],
                )

            # w chain leads on the scalar ring: w0 and w1 alone (the first
            # accumulation chain is gated on them), then pairs.
            w_sb = wpool.tile([P, KT, OPP], mm_dt)
            wf = w_sb.rearrange("p k c -> p (k c)")
            for lo, hi in ((0, 1), (1, 2), (2, 4), (4, 6), (6, 8)):
                nc.scalar.dma_start(
                    out=wf[:, lo * OPP : hi * OPP], in_=w[:, lo * OPP : hi * OPP]
                )
            bias_sb = bpool.tile([P, OPP], mybir.dt.float32)
            nc.scalar.dma_start(out=bias_sb[:], in_=bias[:])

            # supers 1..7: one big contiguous DMA each on the sync ring.
            xs_sb = [None]
            for s in range(1, NSUP):
                t = xpool.tile([P, KT, TW], mm_dt, tag="x")
                nc.sync.dma_start(
                    out=t.rearrange("p k t -> p (k t)"), in_=x[s]
                )
                xs_sb.append(t)

            # ---- super 0: availability-ordered (k, j) over all 8 banks.
            # (The 8th accumulator recycles the warm-up bank; the WAW dep on
            # the in-order PE is free.)
            ps0 = [
                pspool.tile([P, OPP], mybir.dt.float32, name="ps")
                for j in range(TPS)
            ]
            o_sb = opool.tile([P, TPS, OPP], mm_dt, tag="o")
            first = True
            for k, j in _super0_order():
                nc.tensor.matmul(
                    ps0[j][:],
                    x0_sb[:, j, k, :],
                    w_sb[:, k, :],
                    start=(k == 0),
                    stop=(k == KT - 1),
                )
                if first:
                    # ramp fillers: gated on the same first pieces, so they
                    # execute inside the later pieces' DMA-wait holes.
                    for _ in range(NFILL):
                        nc.tensor.matmul(
                            warm_ps[:, 0:P],
                            x0_sb[:, 0, 0, :],
                            w_sb[:, 0, 0:P],
                            start=True,
                            stop=True,
                        )
                    first = False
            for j in range(TPS):
                nc.vector.tensor_add(o_sb[:, j, :], ps0[j][:], bias_sb[:])
            nc.scalar.dma_start(
                out=y[:, 0 : TPS * OPP],
                in_=o_sb.rearrange("p j c -> p (j c)"),
            )

            # ---- supers 1..6: j-groups, per-super store.
            for s in range(1, NSUP - 1):
                x_sb = xs_sb[s]
                o_sb = opool.tile([P, TPS, OPP], mm_dt, tag="o")
                for j in range(TPS):
                    ps = pspool.tile([P, OPP], mybir.dt.float32, name="ps")
                    for k in range(KT):
                        nc.tensor.matmul(
                            ps[:],
                            x_sb[:, k, j * P : (j + 1) * P],
                            w_sb[:, k, :],
                            start=(k == 0),
                            stop=(k == KT - 1),
                        )
                    nc.vector.tensor_add(o_sb[:, j, :], ps[:], bias_sb[:])
                nc.scalar.dma_start(
                    out=y[:, s * TPS * OPP : (s + 1) * TPS * OPP],
                    in_=o_sb.rearrange("p j c -> p (j c)"),
                )

            # ---- super 7: per-group stores; final group split in 2x256 cols
            # so the very last store is small and leaves early.
            # Stores go on the sync ring here: it is idle by now, and the
            # last store must not queue behind scalar-ring issue latency.
            s = NSUP - 1
            x_sb = xs_sb[s]
            for j in range(TPS):
                og = ogpool.tile([P, OPP], mm_dt, tag="og")
                ps = pspool.tile([P, OPP], mybir.dt.float32, name="ps")
                for k in range(KT):
                    nc.tensor.matmul(
                        ps[:],
                        x_sb[:, k, j * P : (j + 1) * P],
                        w_sb[:, k, :],
                        start=(k == 0),
                        stop=(k == KT - 1),
                    )
                nc.vector.tensor_add(og[:], ps[:], bias_sb[:])
                nc.sync.dma_start(
                    out=y[:, YG0 + j * OPP : YG0 + (j + 1) * OPP], in_=og[:]
                )

    nc.compile()
    return nc, np_in


def _get_program(variant: str):
    if variant not in _programs:
        _programs[variant] = _build(variant)
    return _programs[variant]


def kernel(input_, idx_list, W, b, **_ignored):
    from concourse.bass_utils import run_bass_kernel_spmd

    input_ = np.asarray(input_)
    idx = np.asarray(idx_list).astype(np.int64)
    W = np.asarray(W, dtype=np.float32)
    b = np.asarray(b, dtype=np.float32)

    nc, np_in = _get_program(VARIANT)

    in_maps = []
    for e in range(E):
        xg = input_[idx[e]].reshape(TOK, D).astype(np.float32, copy=False)
        xt = np.ascontiguousarray(xg.T).astype(np_in)  # [D, TOK]
        a = xt.reshape(KT, P, NSUP, TW)
        xpk = np.empty((NSUP, P, KT * TW), dtype=np_in)
        # super 0 -> [p][j][k][u]
        xpk[0] = (
            a[:, :, 0].reshape(KT, P, TPS, P).transpose(1, 2, 0, 3)
            .reshape(P, KT * TW)
        )
        # supers 1..7 -> [p][k][t]
        xpk[1:] = a.transpose(2, 1, 0, 3)[1:].reshape(NSUP - 1, P, KT * TW)
        # w -> [p][k][c]
        wt = np.ascontiguousarray(
            W[e].T.reshape(KT, P, OPP).transpose(1, 0, 2)
        ).astype(np_in).reshape(P, KT * OPP)
        bias_bc = np.ascontiguousarray(
            np.broadcast_to(b[e][None, :], (P, OPP))
        ).astype(np.float32)
        in_maps.append({"x": xpk, "w": wt, "bias": bias_bc})

    res = run_bass_kernel_spmd(nc, in_maps, core_ids=list(range(E)))

    out = np.zeros((BS, S, E * OPP), dtype=input_.dtype)
    for e in range(E):
        yw = np.asarray(res.results[e]["y"])  # [P, NSUP*TPS*OPP] bf16
        y_main = (
            yw[:, : (NSUP - 1) * TPS * OPP]
            .reshape(P, NSUP - 1, TPS, OPP)
            .transpose(1, 2, 0, 3)
            .reshape((NSUP - 1) * TW, OPP)
        )
        y_last = (
            yw[:, (NSUP - 1) * TPS * OPP :]
            .reshape(P, TPS, OPP)
            .transpose(1, 0, 2)
            .reshape(TW, OPP)
        )
        ye = np.concatenate([y_main, y_last], axis=0).astype(np.float32)
        out[idx[e], :, e * OPP : (e + 1) * OPP] = ye.reshape(BS // E, S, OPP)
    return out
